# revision 29
# baseline (speedup 1.0000x reference)
"""Multi-head attention (no mask) on 8 trn2 NeuronCores.

Problem: x[4,2048,1024] @ w_attn[1024,3072] + b_attn -> qkv, 16 heads x 64,
softmax(q k^T / 8) v, merge heads, @ w_proj[1024,1024] + b_proj.

Sharding: core c = (batch b = c//2, head-group g = c%2).  Each core handles
one batch and 8 heads (tensor-parallel over heads), producing a partial
c_proj output; the host adds the two partials per batch plus b_proj.

Device layout (all fp32):
  xT   [C, T]     host-pretransposed activation (c on partitions on chip)
  qT,kT[512, T]   = (x @ w_q/k + b)^T, stored as 4 chunks of [128, T]
                   (each chunk = 2 heads stacked 64+64 on partitions)
  v    [T, 512]   natural layout, 16 chunks of [128, 512]
  S^T  [j, i]     per head via row-tiled matmuls (k^T stationary)
  exp  on ACT with fused 1/8 scale, no max subtraction (scores are O(5))
  den  = sum_j exp via ones-matmuls (col-tiled into 32-row PSUM strips)
  y^T  [d, i]     via v-stationary col-tiled matmuls (2 heads concurrent)
  out  [T, 1024]  = y^T.T @ w_proj chunks, accumulated over 4 dcat chunks
"""

import numpy as np
from contextlib import ExitStack

import concourse.bass as bass
import concourse.tile as tile
from concourse import bacc, mybir
from concourse.bass_utils import run_bass_kernel_spmd

F32 = mybir.dt.float32
EXP = mybir.ActivationFunctionType.Exp

B, T_FULL, C = 4, 2048, 1024
N_HEAD, HEAD_DIM = 16, 64
HPG = 8           # heads per group (per core)
QKD = HPG * HEAD_DIM   # 512: per-core q/k/v width
N_CORES = 8
SCALE = 1.0 / np.sqrt(HEAD_DIM)

# test.py can flip these to get a profile out of the run
TRACE = False
LAST_RESULTS = None


def build_bass(T=T_FULL, use_bf16=True):
    """Build the per-core Bass program (same program for all 8 cores)."""
    NCC = C // 128          # 8 c-chunks
    NTC = T // 128          # t-chunks (16 at full size)
    TH_SIZE = T // 2        # phase-1 t-half
    NI_TH = TH_SIZE // 512 if TH_SIZE >= 512 else 1   # 512-col mm splits
    I_BLK = min(512, T)
    N_I = T // I_BLK        # i-blocks (4 at full size)
    N_PAIR = HPG // 2       # 4 head pairs

    # Bacc (not raw Bass): its compile() runs generate_event_semaphores,
    # which legalizes multi-wait instructions (HW allows 1 wait/inst).
    nc = bacc.Bacc("TRN2", target_bir_lowering=False, debug=False,
                   num_devices=N_CORES)

    mdt = mybir.dt.bfloat16 if use_bf16 else F32
    xT = nc.dram_tensor("xT", [C, T], mdt, kind="ExternalInput").ap()
    w_qk = nc.dram_tensor("w_qk", [C, 2 * QKD], mdt, kind="ExternalInput").ap()
    w_v = nc.dram_tensor("w_v", [C, QKD], mdt, kind="ExternalInput").ap()
    b_qk = nc.dram_tensor("b_qk", [128, 8], F32, kind="ExternalInput").ap()
    b_v_bc = nc.dram_tensor("b_v_bc", [128, QKD], F32, kind="ExternalInput").ap()
    w_pr = nc.dram_tensor("w_pr", [QKD, C], mdt, kind="ExternalInput").ap()
    ones = nc.dram_tensor("ones", [128, 64], F32, kind="ExternalInput").ap()
    out = nc.dram_tensor("out", [T, C], F32, kind="ExternalOutput").ap()
    # DRAM bounce for softmax reciprocals (SBUF sources cannot
    # partition-broadcast, DRAM sources can)
    N_I_ = T // min(512, T)
    rcd = nc.dram_tensor("rc_scratch", [HPG // 2, 2 * N_I_, min(512, T)],
                         F32).ap()

    BF = mybir.dt.bfloat16
    edt = BF if use_bf16 else F32

    with tile.TileContext(nc) as tc, ExitStack() as ctx:
        persist = ctx.enter_context(tc.tile_pool(name="persist", bufs=1))
        qT = persist.tile([128, N_PAIR, T], edt)
        kT = persist.tile([128, N_PAIR, T], edt)
        # v stored 65-wide per head: 64 data cols + a ones column that
        # makes row 64 of each y matmul the softmax denominator
        v = persist.tile([128, NTC, HPG * 65], edt)
        ones_sb = persist.tile([128, 64], edt)
        bqk_sb = persist.tile([128, 8], F32)
        bvbc_sb = persist.tile([128, QKD], F32)

        dvescr = persist.tile([1, 8], F32)
        nc.sync.dma_start(out=bqk_sb[:], in_=b_qk)
        nc.sync.dma_start(out=bvbc_sb[:], in_=b_v_bc)
        # DVE-side fences: TT/TS instructions also hold only one sync
        # wait, so absorb each bias-DMA wait into a tiny copy first
        nc.vector.tensor_copy(dvescr[0:1, 0:1], bqk_sb[0:1, 0:1])
        nc.vector.tensor_copy(dvescr[0:1, 1:2], bvbc_sb[0:1, 0:1])
        if use_bf16:
            nc.gpsimd.dma_start(out=ones_sb[:], in_=ones)  # casts f32->bf16
            ones_f32 = persist.tile([128, 64], F32)
            nc.sync.dma_start(out=ones_f32[:], in_=ones)
        else:
            nc.sync.dma_start(out=ones_sb[:], in_=ones)
            ones_f32 = ones_sb

        yT = persist.tile([128, N_PAIR, T], edt)
        wp_sb = persist.tile([128, N_PAIR, C], edt)

        # ---------------- phase 1a: loads + v projection ----------------
        # Fences: each freshly-DMA'd matmul input gets a 1x1x1 dummy
        # matmul so real matmuls see at most one unobserved semaphore
        # (keeps Bacc's event-semaphore splitting to a minimum).
        xT_r = xT.rearrange("(c p) t -> p c t", p=128)
        with tc.tile_pool(name="ph1w", bufs=1) as ph1w, \
             tc.tile_pool(name="ph1x", bufs=2) as ph1x:
            xt0 = ph1x.tile([128, NCC, TH_SIZE], edt, tag="xt")
            xt1 = ph1x.tile([128, NCC, TH_SIZE], edt, tag="xt")
            wqk_sb = ph1w.tile([128, NCC, 2 * QKD], edt)
            wv_sb = ph1w.tile([128, NCC, QKD], edt)
            nc.sync.dma_start(out=wqk_sb[:],
                              in_=w_qk.rearrange("(c p) n -> p c n", p=128))
            nc.sync.dma_start(out=xt0[:], in_=xT_r[:, :, 0:TH_SIZE])
            nc.sync.dma_start(out=wv_sb[:],
                              in_=w_v.rearrange("(c p) n -> p c n", p=128))
            nc.sync.dma_start(out=xt1[:], in_=xT_r[:, :, TH_SIZE:T])
            nc.sync.dma_start(out=wp_sb[:],
                              in_=w_pr.rearrange("(d p) n -> p d n", p=128))
            xts = [xt0, xt1]
            v_r = v[:, :, :].rearrange("q t (h e) -> q t h e", e=65)
            nc.vector.memset(v_r[:, :, :, 64:65], 1.0)

            # v projection is emitted per-tc inside pair 0's first
            # j-loop (tc == j there), so exp can start ~30us earlier.
            def v_proj_tc(tg, ps_pool):
                psv = ps_pool.tile([128, QKD], F32, tag="ps")
                th = tg // (TH_SIZE // 128)
                tcl = tg % (TH_SIZE // 128)
                for c in range(NCC):
                    nc.tensor.matmul(
                        psv[:],
                        xts[th][:, c, tcl * 128:(tcl + 1) * 128],
                        wv_sb[:, c, :],
                        start=(c == 0), stop=(c == NCC - 1))
                nc.vector.tensor_add(
                    v[:, tg, :].rearrange(
                        "q (h e) -> q h e", e=65)[:, :, 0:64],
                    psv[:].rearrange("q (h e) -> q h e", e=64),
                    bvbc_sb[:].rearrange("q (h e) -> q h e", e=64))

            # ------- phase 1b/2: per-pair q/k projection + attention -------
            # Emitting each pair's q/k projection right before its
            # attention lets the scheduler fill PE idle slots (attention
            # is exp/ACT-paced) with the next pair's projection matmuls.
            def qk_proj(p, qk_pool):
                for dcq in (p, p + 4):
                    for th in range(2):
                        for i2 in range(NI_TH):
                            lo = th * TH_SIZE + i2 * 512
                            w = min(512, TH_SIZE)
                            isl = slice(i2 * 512, i2 * 512 + w)
                            ps = qk_pool.tile([128, 512], F32, tag="ps")
                            for c in range(NCC):
                                nc.tensor.matmul(
                                    ps[:, 0:w],
                                    wqk_sb[:, c, dcq * 128:(dcq + 1) * 128],
                                    xts[th][:, c, isl],
                                    start=(c == 0), stop=(c == NCC - 1))
                            dst = (qT if dcq < 4 else kT)[:, p, lo:lo + w]
                            nc.vector.tensor_scalar_add(
                                dst, ps[:, 0:w], bqk_sb[:, dcq:dcq + 1])

        # ---------------- phase 2: attention ----------------
        # Per head: S^T via K=64 matmuls (row-pair per es grain), then
        # y accumulation with M=65 single-tile matmuls whose 65th lhsT
        # column is all-ones -> row 64 of the y accumulator is the
        # softmax denominator (free: matmul time is N-bound).
            with tc.tile_pool(name="pp_qk", bufs=2, space="PSUM") as qk_pool, \
                 tc.tile_pool(name="att_s", bufs=2, space="PSUM") as s_pool, \
                 tc.tile_pool(name="att_y", bufs=2, space="PSUM") as y_pool, \
                 tc.tile_pool(name="att_es", bufs=4) as es_pool, \
                 tc.tile_pool(name="att_yr", bufs=4) as yr_pool, \
                 tc.tile_pool(name="att_st", bufs=4) as st_pool, \
                 tc.tile_pool(name="att_cl", bufs=3) as cl_pool, \
                 tc.tile_pool(name="att_rc", bufs=3) as rc_pool, \
                 tc.tile_pool(name="att_bc", bufs=4) as bc_pool, \
                 tc.tile_pool(name="ph3o", bufs=4) as os_pool:
                fence_ps = y_pool.tile([1, 8], F32, tag="y")
                nc.tensor.matmul(fence_ps[0:1, 0:1], ones_sb[0:1, 0:1],
                                 ones_sb[0:1, 0:1], start=True, stop=True)
                if use_bf16:
                    nc.tensor.matmul(fence_ps[0:1, 1:2], ones_f32[0:1, 0:1],
                                     ones_f32[0:1, 0:1], start=True, stop=True)
                for fi, ft in enumerate((xt0, wv_sb, xt1, wqk_sb, wp_sb)):
                    nc.tensor.matmul(fence_ps[0:1, 2 + fi:3 + fi],
                                     ft[0:1, 0, 0:1], ones_sb[0:1, 0:1],
                                     start=True, stop=True)
                for p in range(N_PAIR):
                    qk_proj(p, qk_pool)
                    yraw_a = yr_pool.tile([64, T], edt, tag="yraw")
                    yraw_b = yr_pool.tile([64, T], edt, tag="yraw")
                    yraws = [yraw_a, yraw_b]
                    for i in range(N_I):
                        isl = slice(i * I_BLK, (i + 1) * I_BLK)
                        y_a = y_pool.tile([65, I_BLK], F32, tag="y")
                        y_b = y_pool.tile([65, I_BLK], F32, tag="y")
                        ys = [y_a, y_b]
                        for j in range(NTC):
                            jsl = slice(j * 128, (j + 1) * 128)
                            s = s_pool.tile([128, 2 * I_BLK], F32, tag="s")
                            nc.tensor.matmul(s[:, 0:I_BLK],
                                             kT[0:64, p, jsl], qT[0:64, p, isl],
                                             start=True, stop=True)
                            nc.tensor.matmul(s[:, I_BLK:2 * I_BLK],
                                             kT[64:128, p, jsl],
                                             qT[64:128, p, isl],
                                             start=True, stop=True)
                            if p == 0 and i == 0:
                                v_proj_tc(j, qk_pool)
                            es = es_pool.tile([128, 2 * I_BLK], edt, tag="es")
                            nc.scalar.activation(es[:], s[:], EXP, scale=SCALE)
                            for hl in range(2):
                                h = 2 * p + hl
                                nc.tensor.matmul(
                                    ys[hl][0:65, :],
                                    v[:, j, 65 * h:65 * h + 65],
                                    es[:, hl * I_BLK:(hl + 1) * I_BLK],
                                    start=(j == 0), stop=(j == NTC - 1))
                        # per-i denominator handling so yT streams out
                        # (keeps the output projection from piling up at
                        # the very end of attention)
                        coll = cl_pool.tile([2, I_BLK], F32, tag="coll")
                        for hl in range(2):
                            st = st_pool.tile([65, I_BLK], F32, tag="st")
                            nc.vector.tensor_copy(st[64:65, :],
                                                  ys[hl][64:65, :])
                            nc.vector.tensor_copy(yraws[hl][:, isl],
                                                  ys[hl][0:64, :])
                            # move denominator row to its own partition
                            nc.gpsimd.dma_start(out=coll[hl:hl + 1, :],
                                                in_=st[64:65, :])
                        rc = rc_pool.tile([2, I_BLK], F32, tag="rc")
                        nc.vector.reciprocal_approx_fast(rc[:], coll[:])
                        nc.gpsimd.dma_start(out=rcd[p, 2 * i:2 * i + 2],
                                            in_=rc[:])
                        for hl in range(2):
                            bcast = bc_pool.tile([64, I_BLK], edt, tag="bcast")
                            rrow = rcd[p, 2 * i + hl, :]
                            rbc = bass.AP(tensor=rrow.tensor,
                                          offset=rrow.offset,
                                          ap=[[0, 64]] + list(rrow.ap))
                            nc.gpsimd.dma_start(out=bcast[:], in_=rbc)
                            if hl == 0:
                                nc.vector.tensor_mul(yT[0:64, p, isl],
                                                     yraws[0][:, isl],
                                                     bcast[:])
                            else:
                                ybst = bc_pool.tile([64, I_BLK], edt,
                                                    tag="ybst")
                                nc.vector.tensor_mul(ybst[:],
                                                     yraws[1][:, isl],
                                                     bcast[:])
                                nc.gpsimd.dma_start(out=yT[64:128, p, isl],
                                                    in_=ybst[:])
                        if p == N_PAIR - 1:
                            # output projection for the t-rows this i-block
                            # completed (all pairs' yT now final there);
                            # reuses the now-idle qk psum slots
                            for tcl in range(4 * i, min(4 * (i + 1), NTC)):
                                for n2 in range(C // 512):
                                    nsl = slice(n2 * 512, (n2 + 1) * 512)
                                    pso = qk_pool.tile([128, 512], F32,
                                                       tag="ps")
                                    for d in range(N_PAIR):
                                        nc.tensor.matmul(
                                            pso[:],
                                            yT[:, d,
                                               tcl * 128:(tcl + 1) * 128],
                                            wp_sb[:, d, nsl],
                                            start=(d == 0),
                                            stop=(d == N_PAIR - 1))
                                    os = os_pool.tile([128, 512], F32,
                                                      tag="os")
                                    # absorb WAR on the slot's prior out-DMA
                                    nc.vector.memset(os[0:1, 0:1], 0.0)
                                    nc.vector.tensor_copy(os[:], pso[:])
                                    nc.sync.dma_start(
                                        out=out[tcl * 128:(tcl + 1) * 128,
                                                nsl],
                                        in_=os[:])

    nc.compile()
    return nc


def make_in_maps(x, w_attn, b_attn, w_proj, T=T_FULL, use_bf16=True):
    """Host-side sharding: per-core input dict."""
    import ml_dtypes
    mdt = ml_dtypes.bfloat16 if use_bf16 else np.float32
    x = np.asarray(x, dtype=np.float32)
    w_attn = np.asarray(w_attn, dtype=np.float32)
    b_attn = np.asarray(b_attn, dtype=np.float32)
    w_proj = np.asarray(w_proj, dtype=np.float32)
    in_maps = []
    ones = np.ones((128, 64), dtype=np.float32)
    for core in range(N_CORES):
        b, g = core // 2, core % 2
        gq = slice(g * QKD, (g + 1) * QKD)
        gk = slice(C + g * QKD, C + (g + 1) * QKD)
        gv = slice(2 * C + g * QKD, 2 * C + (g + 1) * QKD)
        w_qk = np.concatenate([w_attn[:, gq], w_attn[:, gk]], axis=1)
        b_q = b_attn[gq]
        b_k = b_attn[gk]
        b_v = b_attn[gv]
        b_qk = np.stack([b_q.reshape(4, 128), b_k.reshape(4, 128)],
                        axis=0).reshape(8, 128).T.copy()   # [128, 8]
        in_maps.append({
            "xT": np.ascontiguousarray(x[b, :T].T).astype(mdt),
            "w_qk": np.ascontiguousarray(w_qk).astype(mdt),
            "w_v": np.ascontiguousarray(w_attn[:, gv]).astype(mdt),
            "b_qk": np.ascontiguousarray(b_qk),
            "b_v_bc": np.tile(b_v, (128, 1)),
            "w_pr": np.ascontiguousarray(w_proj[gq]).astype(mdt),
            "ones": ones,
        })
    return in_maps


def kernel(x, w_attn, b_attn, w_proj, b_proj):
    global LAST_RESULTS
    in_maps = make_in_maps(x, w_attn, b_attn, w_proj)
    nc = build_bass()
    res = run_bass_kernel_spmd(nc, in_maps, list(range(N_CORES)), trace=TRACE)
    LAST_RESULTS = res
    b_proj = np.asarray(b_proj, dtype=np.float32)
    out = np.empty((B, T_FULL, C), dtype=np.float32)
    for b in range(B):
        out[b] = res.results[2 * b]["out"] + res.results[2 * b + 1]["out"] \
            + b_proj[None, :]
    return out


# revision 31
# speedup vs baseline: 1.0263x; 1.0263x over previous
"""Multi-head attention (no mask) on 8 trn2 NeuronCores.

Problem: x[4,2048,1024] @ w_attn[1024,3072] + b_attn -> qkv, 16 heads x 64,
softmax(q k^T / 8) v, merge heads, @ w_proj[1024,1024] + b_proj.

Sharding: core c = (batch b = c//2, head-group g = c%2).  Each core handles
one batch and 8 heads (tensor-parallel over heads), producing a partial
c_proj output; the host adds the two partials per batch plus b_proj.

Device layout (all fp32):
  xT   [C, T]     host-pretransposed activation (c on partitions on chip)
  qT,kT[512, T]   = (x @ w_q/k + b)^T, stored as 4 chunks of [128, T]
                   (each chunk = 2 heads stacked 64+64 on partitions)
  v    [T, 512]   natural layout, 16 chunks of [128, 512]
  S^T  [j, i]     per head via row-tiled matmuls (k^T stationary)
  exp  on ACT with fused 1/8 scale, no max subtraction (scores are O(5))
  den  = sum_j exp via ones-matmuls (col-tiled into 32-row PSUM strips)
  y^T  [d, i]     via v-stationary col-tiled matmuls (2 heads concurrent)
  out  [T, 1024]  = y^T.T @ w_proj chunks, accumulated over 4 dcat chunks
"""

import numpy as np
from contextlib import ExitStack

import concourse.bass as bass
import concourse.tile as tile
from concourse import bacc, mybir
from concourse.bass_utils import run_bass_kernel_spmd

F32 = mybir.dt.float32
EXP = mybir.ActivationFunctionType.Exp

B, T_FULL, C = 4, 2048, 1024
N_HEAD, HEAD_DIM = 16, 64
HPG = 8           # heads per group (per core)
QKD = HPG * HEAD_DIM   # 512: per-core q/k/v width
N_CORES = 8
SCALE = 1.0 / np.sqrt(HEAD_DIM)

# test.py can flip these to get a profile out of the run
TRACE = False
LAST_RESULTS = None


def build_bass(T=T_FULL, use_bf16=True):
    """Build the per-core Bass program (same program for all 8 cores)."""
    NCC = C // 128          # 8 c-chunks
    NTC = T // 128          # t-chunks (16 at full size)
    TH_SIZE = T // 2        # phase-1 t-half
    NI_TH = TH_SIZE // 512 if TH_SIZE >= 512 else 1   # 512-col mm splits
    I_BLK = min(512, T)
    N_I = T // I_BLK        # i-blocks (4 at full size)
    N_PAIR = HPG // 2       # 4 head pairs

    # Bacc (not raw Bass): its compile() runs generate_event_semaphores,
    # which legalizes multi-wait instructions (HW allows 1 wait/inst).
    nc = bacc.Bacc("TRN2", target_bir_lowering=False, debug=False,
                   num_devices=N_CORES)

    mdt = mybir.dt.bfloat16 if use_bf16 else F32
    xT = nc.dram_tensor("xT", [C, T], mdt, kind="ExternalInput").ap()
    w_qk = nc.dram_tensor("w_qk", [C, 2 * QKD], mdt, kind="ExternalInput").ap()
    w_v = nc.dram_tensor("w_v", [C, QKD], mdt, kind="ExternalInput").ap()
    b_qk = nc.dram_tensor("b_qk", [128, 8], F32, kind="ExternalInput").ap()
    b_v_bc = nc.dram_tensor("b_v_bc", [128, QKD], F32, kind="ExternalInput").ap()
    w_pr = nc.dram_tensor("w_pr", [QKD, C], mdt, kind="ExternalInput").ap()
    ones = nc.dram_tensor("ones", [128, 64], F32, kind="ExternalInput").ap()
    out = nc.dram_tensor("out", [T, C], F32, kind="ExternalOutput").ap()
    # DRAM bounce for softmax reciprocals (SBUF sources cannot
    # partition-broadcast, DRAM sources can)
    N_I_ = T // min(512, T)
    rcd = nc.dram_tensor("rc_scratch", [HPG // 2, 2 * N_I_, min(512, T)],
                         F32).ap()

    BF = mybir.dt.bfloat16
    edt = BF if use_bf16 else F32

    with tile.TileContext(nc) as tc, ExitStack() as ctx:
        persist = ctx.enter_context(tc.tile_pool(name="persist", bufs=1))
        qT = persist.tile([128, N_PAIR, T], edt)
        kT = persist.tile([128, N_PAIR, T], edt)
        # v stored 65-wide per head: 64 data cols + a ones column that
        # makes row 64 of each y matmul the softmax denominator
        v = persist.tile([128, NTC, HPG * 65], edt)
        ones_sb = persist.tile([128, 64], edt)
        bqk_sb = persist.tile([128, 8], F32)
        bvbc_sb = persist.tile([128, QKD], F32)

        dvescr = persist.tile([1, 8], F32)
        nc.sync.dma_start(out=bqk_sb[:], in_=b_qk)
        nc.sync.dma_start(out=bvbc_sb[:], in_=b_v_bc)
        # DVE-side fences: TT/TS instructions also hold only one sync
        # wait, so absorb each bias-DMA wait into a tiny copy first
        nc.vector.tensor_copy(dvescr[0:1, 0:1], bqk_sb[0:1, 0:1])
        nc.vector.tensor_copy(dvescr[0:1, 1:2], bvbc_sb[0:1, 0:1])
        if use_bf16:
            nc.gpsimd.dma_start(out=ones_sb[:], in_=ones)  # casts f32->bf16
            ones_f32 = persist.tile([128, 64], F32)
            nc.sync.dma_start(out=ones_f32[:], in_=ones)
        else:
            nc.sync.dma_start(out=ones_sb[:], in_=ones)
            ones_f32 = ones_sb

        yT = persist.tile([128, N_PAIR, T], edt)
        wp_sb = persist.tile([128, N_PAIR, C], edt)

        # ---------------- phase 1a: loads + v projection ----------------
        # Fences: each freshly-DMA'd matmul input gets a 1x1x1 dummy
        # matmul so real matmuls see at most one unobserved semaphore
        # (keeps Bacc's event-semaphore splitting to a minimum).
        xT_r = xT.rearrange("(c p) t -> p c t", p=128)
        with tc.tile_pool(name="ph1w", bufs=1) as ph1w, \
             tc.tile_pool(name="ph1x", bufs=2) as ph1x:
            xt0 = ph1x.tile([128, NCC, TH_SIZE], edt, tag="xt")
            xt1 = ph1x.tile([128, NCC, TH_SIZE], edt, tag="xt")
            wqk_sb = ph1w.tile([128, NCC, 2 * QKD], edt)
            wv_sb = ph1w.tile([128, NCC, QKD], edt)
            nc.sync.dma_start(out=wqk_sb[:],
                              in_=w_qk.rearrange("(c p) n -> p c n", p=128))
            nc.sync.dma_start(out=xt0[:], in_=xT_r[:, :, 0:TH_SIZE])
            nc.sync.dma_start(out=wv_sb[:],
                              in_=w_v.rearrange("(c p) n -> p c n", p=128))
            nc.sync.dma_start(out=xt1[:], in_=xT_r[:, :, TH_SIZE:T])
            nc.sync.dma_start(out=wp_sb[:],
                              in_=w_pr.rearrange("(d p) n -> p d n", p=128))
            xts = [xt0, xt1]
            v_r = v[:, :, :].rearrange("q t (h e) -> q t h e", e=65)
            nc.vector.memset(v_r[:, :, :, 64:65], 1.0)

            # v projection is emitted per-tc inside pair 0's first
            # j-loop (tc == j there), so exp can start ~30us earlier.
            def v_proj_tc(tg, ps_pool):
                psv = ps_pool.tile([128, QKD], F32, tag="ps")
                th = tg // (TH_SIZE // 128)
                tcl = tg % (TH_SIZE // 128)
                for c in range(NCC):
                    nc.tensor.matmul(
                        psv[:],
                        xts[th][:, c, tcl * 128:(tcl + 1) * 128],
                        wv_sb[:, c, :],
                        start=(c == 0), stop=(c == NCC - 1))
                nc.vector.tensor_add(
                    v[:, tg, :].rearrange(
                        "q (h e) -> q h e", e=65)[:, :, 0:64],
                    psv[:].rearrange("q (h e) -> q h e", e=64),
                    bvbc_sb[:].rearrange("q (h e) -> q h e", e=64))

            # ------- phase 1b/2: per-pair q/k projection + attention -------
            # Emitting each pair's q/k projection right before its
            # attention lets the scheduler fill PE idle slots (attention
            # is exp/ACT-paced) with the next pair's projection matmuls.
            def qk_proj(p, qk_pool):
                for dcq in (p, p + 4):
                    for th in range(2):
                        for i2 in range(NI_TH):
                            lo = th * TH_SIZE + i2 * 512
                            w = min(512, TH_SIZE)
                            isl = slice(i2 * 512, i2 * 512 + w)
                            ps = qk_pool.tile([128, 512], F32, tag="ps")
                            for c in range(NCC):
                                nc.tensor.matmul(
                                    ps[:, 0:w],
                                    wqk_sb[:, c, dcq * 128:(dcq + 1) * 128],
                                    xts[th][:, c, isl],
                                    start=(c == 0), stop=(c == NCC - 1))
                            dst = (qT if dcq < 4 else kT)[:, p, lo:lo + w]
                            nc.vector.tensor_scalar_add(
                                dst, ps[:, 0:w], bqk_sb[:, dcq:dcq + 1])

        # ---------------- phase 2: attention ----------------
        # Per head: S^T via K=64 matmuls (row-pair per es grain), then
        # y accumulation with M=65 single-tile matmuls whose 65th lhsT
        # column is all-ones -> row 64 of the y accumulator is the
        # softmax denominator (free: matmul time is N-bound).
            with tc.tile_pool(name="pp_qk", bufs=2, space="PSUM") as qk_pool, \
                 tc.tile_pool(name="att_s", bufs=2, space="PSUM") as s_pool, \
                 tc.tile_pool(name="att_y", bufs=2, space="PSUM") as y_pool, \
                 tc.tile_pool(name="att_es", bufs=4) as es_pool, \
                 tc.tile_pool(name="att_yr", bufs=4) as yr_pool, \
                 tc.tile_pool(name="att_st", bufs=4) as st_pool, \
                 tc.tile_pool(name="att_cl", bufs=3) as cl_pool, \
                 tc.tile_pool(name="att_rc", bufs=3) as rc_pool, \
                 tc.tile_pool(name="att_bc", bufs=4) as bc_pool:
                fence_ps = y_pool.tile([1, 8], F32, tag="y")
                nc.tensor.matmul(fence_ps[0:1, 0:1], ones_sb[0:1, 0:1],
                                 ones_sb[0:1, 0:1], start=True, stop=True)
                if use_bf16:
                    nc.tensor.matmul(fence_ps[0:1, 1:2], ones_f32[0:1, 0:1],
                                     ones_f32[0:1, 0:1], start=True, stop=True)
                for fi, ft in enumerate((xt0, wv_sb, xt1, wqk_sb, wp_sb)):
                    nc.tensor.matmul(fence_ps[0:1, 2 + fi:3 + fi],
                                     ft[0:1, 0, 0:1], ones_sb[0:1, 0:1],
                                     start=True, stop=True)
                for p in range(N_PAIR):
                    qk_proj(p, qk_pool)
                    yraw_a = yr_pool.tile([64, T], edt, tag="yraw")
                    yraw_b = yr_pool.tile([64, T], edt, tag="yraw")
                    yraws = [yraw_a, yraw_b]
                    for i in range(N_I):
                        isl = slice(i * I_BLK, (i + 1) * I_BLK)
                        y_a = y_pool.tile([65, I_BLK], F32, tag="y")
                        y_b = y_pool.tile([65, I_BLK], F32, tag="y")
                        ys = [y_a, y_b]
                        for j in range(NTC):
                            jsl = slice(j * 128, (j + 1) * 128)
                            s = s_pool.tile([128, 2 * I_BLK], F32, tag="s")
                            nc.tensor.matmul(s[:, 0:I_BLK],
                                             kT[0:64, p, jsl], qT[0:64, p, isl],
                                             start=True, stop=True)
                            nc.tensor.matmul(s[:, I_BLK:2 * I_BLK],
                                             kT[64:128, p, jsl],
                                             qT[64:128, p, isl],
                                             start=True, stop=True)
                            if p == 0 and i == 0:
                                v_proj_tc(j, qk_pool)
                            es = es_pool.tile([128, 2 * I_BLK], edt, tag="es")
                            nc.scalar.activation(es[:], s[:], EXP, scale=SCALE)
                            for hl in range(2):
                                h = 2 * p + hl
                                nc.tensor.matmul(
                                    ys[hl][0:65, :],
                                    v[:, j, 65 * h:65 * h + 65],
                                    es[:, hl * I_BLK:(hl + 1) * I_BLK],
                                    start=(j == 0), stop=(j == NTC - 1))
                        # per-i denominator handling so yT streams out
                        # (keeps the output projection from piling up at
                        # the very end of attention)
                        coll = cl_pool.tile([2, I_BLK], F32, tag="coll")
                        for hl in range(2):
                            st = st_pool.tile([65, I_BLK], F32, tag="st")
                            nc.vector.tensor_copy(st[64:65, :],
                                                  ys[hl][64:65, :])
                            nc.vector.tensor_copy(yraws[hl][:, isl],
                                                  ys[hl][0:64, :])
                            # move denominator row to its own partition
                            nc.gpsimd.dma_start(out=coll[hl:hl + 1, :],
                                                in_=st[64:65, :])
                        rc = rc_pool.tile([2, I_BLK], F32, tag="rc")
                        nc.vector.reciprocal_approx_fast(rc[:], coll[:])
                        nc.gpsimd.dma_start(out=rcd[p, 2 * i:2 * i + 2],
                                            in_=rc[:])
                        for hl in range(2):
                            bcast = bc_pool.tile([64, I_BLK], edt, tag="bcast")
                            rrow = rcd[p, 2 * i + hl, :]
                            rbc = bass.AP(tensor=rrow.tensor,
                                          offset=rrow.offset,
                                          ap=[[0, 64]] + list(rrow.ap))
                            nc.gpsimd.dma_start(out=bcast[:], in_=rbc)
                            if hl == 0:
                                nc.vector.tensor_mul(yT[0:64, p, isl],
                                                     yraws[0][:, isl],
                                                     bcast[:])
                            else:
                                ybst = bc_pool.tile([64, I_BLK], edt,
                                                    tag="ybst")
                                nc.vector.tensor_mul(ybst[:],
                                                     yraws[1][:, isl],
                                                     bcast[:])
                                nc.gpsimd.dma_start(out=yT[64:128, p, isl],
                                                    in_=ybst[:])

        # ---------------- phase 3: output projection ----------------
        with tc.tile_pool(name="ph3o", bufs=3) as ph3o, \
             tc.tile_pool(name="pp_o", bufs=3, space="PSUM") as pp_o:
            for tcl in range(NTC):
                ps = pp_o.tile([128, C], F32, tag="pso")
                for d in range(N_PAIR):
                    for n2 in range(C // 512):
                        nsl = slice(n2 * 512, (n2 + 1) * 512)
                        nc.tensor.matmul(
                            ps[:, nsl],
                            yT[:, d, tcl * 128:(tcl + 1) * 128],
                            wp_sb[:, d, nsl],
                            start=(d == 0), stop=(d == N_PAIR - 1))
                os = ph3o.tile([128, C], F32, tag="os")
                # absorb the WAR wait on the slot's previous out-DMA
                nc.vector.memset(os[0:1, 0:1], 0.0)
                nc.vector.tensor_copy(os[:], ps[:])
                nc.sync.dma_start(out=out[tcl * 128:(tcl + 1) * 128, :],
                                  in_=os[:])

    nc.compile()
    return nc


def make_in_maps(x, w_attn, b_attn, w_proj, T=T_FULL, use_bf16=True):
    """Host-side sharding: per-core input dict."""
    import ml_dtypes
    mdt = ml_dtypes.bfloat16 if use_bf16 else np.float32
    x = np.asarray(x, dtype=np.float32)
    w_attn = np.asarray(w_attn, dtype=np.float32)
    b_attn = np.asarray(b_attn, dtype=np.float32)
    w_proj = np.asarray(w_proj, dtype=np.float32)
    in_maps = []
    ones = np.ones((128, 64), dtype=np.float32)
    for core in range(N_CORES):
        b, g = core // 2, core % 2
        gq = slice(g * QKD, (g + 1) * QKD)
        gk = slice(C + g * QKD, C + (g + 1) * QKD)
        gv = slice(2 * C + g * QKD, 2 * C + (g + 1) * QKD)
        w_qk = np.concatenate([w_attn[:, gq], w_attn[:, gk]], axis=1)
        b_q = b_attn[gq]
        b_k = b_attn[gk]
        b_v = b_attn[gv]
        b_qk = np.stack([b_q.reshape(4, 128), b_k.reshape(4, 128)],
                        axis=0).reshape(8, 128).T.copy()   # [128, 8]
        in_maps.append({
            "xT": np.ascontiguousarray(x[b, :T].T).astype(mdt),
            "w_qk": np.ascontiguousarray(w_qk).astype(mdt),
            "w_v": np.ascontiguousarray(w_attn[:, gv]).astype(mdt),
            "b_qk": np.ascontiguousarray(b_qk),
            "b_v_bc": np.tile(b_v, (128, 1)),
            "w_pr": np.ascontiguousarray(w_proj[gq]).astype(mdt),
            "ones": ones,
        })
    return in_maps


def kernel(x, w_attn, b_attn, w_proj, b_proj):
    global LAST_RESULTS
    in_maps = make_in_maps(x, w_attn, b_attn, w_proj)
    nc = build_bass()
    try:
        res = run_bass_kernel_spmd(nc, in_maps, list(range(N_CORES)),
                                   trace=TRACE)
    except Exception:
        # rare transient NRT exec-unit errors: one retry
        res = run_bass_kernel_spmd(nc, in_maps, list(range(N_CORES)),
                                   trace=TRACE)
    LAST_RESULTS = res
    b_proj = np.asarray(b_proj, dtype=np.float32)
    out = np.empty((B, T_FULL, C), dtype=np.float32)
    for b in range(B):
        out[b] = res.results[2 * b]["out"] + res.results[2 * b + 1]["out"] \
            + b_proj[None, :]
    return out


# revision 32
# speedup vs baseline: 1.0457x; 1.0189x over previous
"""Multi-head attention (no mask) on 8 trn2 NeuronCores.

Problem: x[4,2048,1024] @ w_attn[1024,3072] + b_attn -> qkv, 16 heads x 64,
softmax(q k^T / 8) v, merge heads, @ w_proj[1024,1024] + b_proj.

Sharding: core c = (batch b = c//2, head-group g = c%2).  Each core handles
one batch and 8 heads (tensor-parallel over heads), producing a partial
c_proj output; the host adds the two partials per batch plus b_proj.

Device layout (all fp32):
  xT   [C, T]     host-pretransposed activation (c on partitions on chip)
  qT,kT[512, T]   = (x @ w_q/k + b)^T, stored as 4 chunks of [128, T]
                   (each chunk = 2 heads stacked 64+64 on partitions)
  v    [T, 512]   natural layout, 16 chunks of [128, 512]
  S^T  [j, i]     per head via row-tiled matmuls (k^T stationary)
  exp  on ACT with fused 1/8 scale, no max subtraction (scores are O(5))
  den  = sum_j exp via ones-matmuls (col-tiled into 32-row PSUM strips)
  y^T  [d, i]     via v-stationary col-tiled matmuls (2 heads concurrent)
  out  [T, 1024]  = y^T.T @ w_proj chunks, accumulated over 4 dcat chunks
"""

import numpy as np
from contextlib import ExitStack

import concourse.bass as bass
import concourse.tile as tile
from concourse import bacc, mybir
from concourse.bass_utils import run_bass_kernel_spmd

F32 = mybir.dt.float32
EXP = mybir.ActivationFunctionType.Exp

B, T_FULL, C = 4, 2048, 1024
N_HEAD, HEAD_DIM = 16, 64
HPG = 8           # heads per group (per core)
QKD = HPG * HEAD_DIM   # 512: per-core q/k/v width
N_CORES = 8
SCALE = 1.0 / np.sqrt(HEAD_DIM)

# test.py can flip these to get a profile out of the run
TRACE = False
LAST_RESULTS = None


def build_bass(T=T_FULL, use_bf16=True):
    """Build the per-core Bass program (same program for all 8 cores)."""
    NCC = C // 128          # 8 c-chunks
    NTC = T // 128          # t-chunks (16 at full size)
    TH_SIZE = T // 2        # phase-1 t-half
    NI_TH = TH_SIZE // 512 if TH_SIZE >= 512 else 1   # 512-col mm splits
    I_BLK = min(512, T)
    N_I = T // I_BLK        # i-blocks (4 at full size)
    N_PAIR = HPG // 2       # 4 head pairs

    # Bacc (not raw Bass): its compile() runs generate_event_semaphores,
    # which legalizes multi-wait instructions (HW allows 1 wait/inst).
    nc = bacc.Bacc("TRN2", target_bir_lowering=False, debug=False,
                   num_devices=N_CORES)

    mdt = mybir.dt.bfloat16 if use_bf16 else F32
    xT = nc.dram_tensor("xT", [C, T], mdt, kind="ExternalInput").ap()
    w_qk = nc.dram_tensor("w_qk", [C, 2 * QKD], mdt, kind="ExternalInput").ap()
    w_v = nc.dram_tensor("w_v", [C, QKD], mdt, kind="ExternalInput").ap()
    b_qk = nc.dram_tensor("b_qk", [128, 8], F32, kind="ExternalInput").ap()
    b_v_bc = nc.dram_tensor("b_v_bc", [128, QKD], F32, kind="ExternalInput").ap()
    w_pr = nc.dram_tensor("w_pr", [QKD, C], mdt, kind="ExternalInput").ap()
    ones = nc.dram_tensor("ones", [128, 64], F32, kind="ExternalInput").ap()
    out = nc.dram_tensor("out", [T, C], F32, kind="ExternalOutput").ap()
    # DRAM bounce for softmax reciprocals (SBUF sources cannot
    # partition-broadcast, DRAM sources can)
    N_I_ = T // min(512, T)
    rcd = nc.dram_tensor("rc_scratch", [HPG // 2, 2 * N_I_, min(512, T)],
                         F32).ap()

    BF = mybir.dt.bfloat16
    edt = BF if use_bf16 else F32

    with tile.TileContext(nc) as tc, ExitStack() as ctx:
        persist = ctx.enter_context(tc.tile_pool(name="persist", bufs=1))
        qT = persist.tile([128, N_PAIR, T], edt)
        kT = persist.tile([128, N_PAIR, T], edt)
        # v stored 65-wide per head: 64 data cols + a ones column that
        # makes row 64 of each y matmul the softmax denominator
        v = persist.tile([128, NTC, HPG * 65], edt)
        ones_sb = persist.tile([128, 64], edt)
        bqk_sb = persist.tile([128, 8], F32)
        bvbc_sb = persist.tile([128, QKD], F32)

        dvescr = persist.tile([1, 8], F32)
        nc.sync.dma_start(out=bqk_sb[:], in_=b_qk)
        nc.sync.dma_start(out=bvbc_sb[:], in_=b_v_bc)
        # DVE-side fences: TT/TS instructions also hold only one sync
        # wait, so absorb each bias-DMA wait into a tiny copy first
        nc.vector.tensor_copy(dvescr[0:1, 0:1], bqk_sb[0:1, 0:1])
        nc.vector.tensor_copy(dvescr[0:1, 1:2], bvbc_sb[0:1, 0:1])
        if use_bf16:
            nc.gpsimd.dma_start(out=ones_sb[:], in_=ones)  # casts f32->bf16
            ones_f32 = persist.tile([128, 64], F32)
            nc.sync.dma_start(out=ones_f32[:], in_=ones)
        else:
            nc.sync.dma_start(out=ones_sb[:], in_=ones)
            ones_f32 = ones_sb

        yT = persist.tile([128, N_PAIR, T], edt)
        wp_sb = persist.tile([128, N_PAIR, C], edt)

        # ---------------- phase 1a: loads + v projection ----------------
        # Fences: each freshly-DMA'd matmul input gets a 1x1x1 dummy
        # matmul so real matmuls see at most one unobserved semaphore
        # (keeps Bacc's event-semaphore splitting to a minimum).
        xT_r = xT.rearrange("(c p) t -> p c t", p=128)
        with tc.tile_pool(name="ph1w", bufs=1) as ph1w, \
             tc.tile_pool(name="ph1x", bufs=2) as ph1x:
            xt0 = ph1x.tile([128, NCC, TH_SIZE], edt, tag="xt")
            xt1 = ph1x.tile([128, NCC, TH_SIZE], edt, tag="xt")
            wqk_sb = ph1w.tile([128, NCC, 2 * QKD], edt)
            wv_sb = ph1w.tile([128, NCC, QKD], edt)
            # pair-0's q/k weight slices land first so the first
            # projection matmuls (and hence exp) start ~25us earlier
            wqk_r = w_qk.rearrange("(c p) n -> p c n", p=128)

            def load_wqk_pair(pp):
                for dcq in (pp, pp + 4):
                    csl = slice(dcq * 128, (dcq + 1) * 128)
                    nc.sync.dma_start(out=wqk_sb[:, :, csl],
                                      in_=wqk_r[:, :, csl])

            load_wqk_pair(0)
            nc.sync.dma_start(out=xt0[:], in_=xT_r[:, :, 0:TH_SIZE])
            nc.sync.dma_start(out=xt1[:], in_=xT_r[:, :, TH_SIZE:T])
            nc.sync.dma_start(out=wv_sb[:],
                              in_=w_v.rearrange("(c p) n -> p c n", p=128))
            for pp in range(1, N_PAIR):
                load_wqk_pair(pp)
            nc.sync.dma_start(out=wp_sb[:],
                              in_=w_pr.rearrange("(d p) n -> p d n", p=128))
            xts = [xt0, xt1]
            v_r = v[:, :, :].rearrange("q t (h e) -> q t h e", e=65)
            nc.vector.memset(v_r[:, :, :, 64:65], 1.0)

            # v projection is emitted per-tc inside pair 0's first
            # j-loop (tc == j there), so exp can start ~30us earlier.
            def v_proj_tc(tg, ps_pool):
                psv = ps_pool.tile([128, QKD], F32, tag="ps")
                th = tg // (TH_SIZE // 128)
                tcl = tg % (TH_SIZE // 128)
                for c in range(NCC):
                    nc.tensor.matmul(
                        psv[:],
                        xts[th][:, c, tcl * 128:(tcl + 1) * 128],
                        wv_sb[:, c, :],
                        start=(c == 0), stop=(c == NCC - 1))
                nc.vector.tensor_add(
                    v[:, tg, :].rearrange(
                        "q (h e) -> q h e", e=65)[:, :, 0:64],
                    psv[:].rearrange("q (h e) -> q h e", e=64),
                    bvbc_sb[:].rearrange("q (h e) -> q h e", e=64))

            # ------- phase 1b/2: per-pair q/k projection + attention -------
            # Emitting each pair's q/k projection right before its
            # attention lets the scheduler fill PE idle slots (attention
            # is exp/ACT-paced) with the next pair's projection matmuls.
            def qk_proj(p, qk_pool):
                for dcq in (p, p + 4):
                    for th in range(2):
                        for i2 in range(NI_TH):
                            lo = th * TH_SIZE + i2 * 512
                            w = min(512, TH_SIZE)
                            isl = slice(i2 * 512, i2 * 512 + w)
                            ps = qk_pool.tile([128, 512], F32, tag="ps")
                            for c in range(NCC):
                                nc.tensor.matmul(
                                    ps[:, 0:w],
                                    wqk_sb[:, c, dcq * 128:(dcq + 1) * 128],
                                    xts[th][:, c, isl],
                                    start=(c == 0), stop=(c == NCC - 1))
                            dst = (qT if dcq < 4 else kT)[:, p, lo:lo + w]
                            nc.vector.tensor_scalar_add(
                                dst, ps[:, 0:w], bqk_sb[:, dcq:dcq + 1])

        # ---------------- phase 2: attention ----------------
        # Per head: S^T via K=64 matmuls (row-pair per es grain), then
        # y accumulation with M=65 single-tile matmuls whose 65th lhsT
        # column is all-ones -> row 64 of the y accumulator is the
        # softmax denominator (free: matmul time is N-bound).
            with tc.tile_pool(name="pp_qk", bufs=2, space="PSUM") as qk_pool, \
                 tc.tile_pool(name="att_s", bufs=2, space="PSUM") as s_pool, \
                 tc.tile_pool(name="att_y", bufs=2, space="PSUM") as y_pool, \
                 tc.tile_pool(name="att_es", bufs=4) as es_pool, \
                 tc.tile_pool(name="att_yr", bufs=4) as yr_pool, \
                 tc.tile_pool(name="att_st", bufs=4) as st_pool, \
                 tc.tile_pool(name="att_cl", bufs=3) as cl_pool, \
                 tc.tile_pool(name="att_rc", bufs=3) as rc_pool, \
                 tc.tile_pool(name="att_bc", bufs=4) as bc_pool:
                fence_ps = y_pool.tile([1, 8], F32, tag="y")
                nc.tensor.matmul(fence_ps[0:1, 0:1], ones_sb[0:1, 0:1],
                                 ones_sb[0:1, 0:1], start=True, stop=True)
                if use_bf16:
                    nc.tensor.matmul(fence_ps[0:1, 1:2], ones_f32[0:1, 0:1],
                                     ones_f32[0:1, 0:1], start=True, stop=True)
                for fi, ft in enumerate((xt0, wv_sb, xt1, wqk_sb, wp_sb)):
                    nc.tensor.matmul(fence_ps[0:1, 2 + fi:3 + fi],
                                     ft[0:1, 0, 0:1], ones_sb[0:1, 0:1],
                                     start=True, stop=True)
                for p in range(N_PAIR):
                    qk_proj(p, qk_pool)
                    yraw_a = yr_pool.tile([64, T], edt, tag="yraw")
                    yraw_b = yr_pool.tile([64, T], edt, tag="yraw")
                    yraws = [yraw_a, yraw_b]
                    for i in range(N_I):
                        isl = slice(i * I_BLK, (i + 1) * I_BLK)
                        y_a = y_pool.tile([65, I_BLK], F32, tag="y")
                        y_b = y_pool.tile([65, I_BLK], F32, tag="y")
                        ys = [y_a, y_b]
                        for j in range(NTC):
                            jsl = slice(j * 128, (j + 1) * 128)
                            s = s_pool.tile([128, 2 * I_BLK], F32, tag="s")
                            nc.tensor.matmul(s[:, 0:I_BLK],
                                             kT[0:64, p, jsl], qT[0:64, p, isl],
                                             start=True, stop=True)
                            nc.tensor.matmul(s[:, I_BLK:2 * I_BLK],
                                             kT[64:128, p, jsl],
                                             qT[64:128, p, isl],
                                             start=True, stop=True)
                            if p == 0 and i == 0:
                                v_proj_tc(j, qk_pool)
                            es = es_pool.tile([128, 2 * I_BLK], edt, tag="es")
                            nc.scalar.activation(es[:], s[:], EXP, scale=SCALE)
                            for hl in range(2):
                                h = 2 * p + hl
                                nc.tensor.matmul(
                                    ys[hl][0:65, :],
                                    v[:, j, 65 * h:65 * h + 65],
                                    es[:, hl * I_BLK:(hl + 1) * I_BLK],
                                    start=(j == 0), stop=(j == NTC - 1))
                        # per-i denominator handling so yT streams out
                        # (keeps the output projection from piling up at
                        # the very end of attention)
                        coll = cl_pool.tile([2, I_BLK], F32, tag="coll")
                        for hl in range(2):
                            st = st_pool.tile([65, I_BLK], F32, tag="st")
                            nc.vector.tensor_copy(st[64:65, :],
                                                  ys[hl][64:65, :])
                            nc.vector.tensor_copy(yraws[hl][:, isl],
                                                  ys[hl][0:64, :])
                            # move denominator row to its own partition
                            nc.gpsimd.dma_start(out=coll[hl:hl + 1, :],
                                                in_=st[64:65, :])
                        rc = rc_pool.tile([2, I_BLK], F32, tag="rc")
                        nc.vector.reciprocal_approx_fast(rc[:], coll[:])
                        nc.gpsimd.dma_start(out=rcd[p, 2 * i:2 * i + 2],
                                            in_=rc[:])
                        for hl in range(2):
                            bcast = bc_pool.tile([64, I_BLK], edt, tag="bcast")
                            rrow = rcd[p, 2 * i + hl, :]
                            rbc = bass.AP(tensor=rrow.tensor,
                                          offset=rrow.offset,
                                          ap=[[0, 64]] + list(rrow.ap))
                            nc.gpsimd.dma_start(out=bcast[:], in_=rbc)
                            if hl == 0:
                                nc.vector.tensor_mul(yT[0:64, p, isl],
                                                     yraws[0][:, isl],
                                                     bcast[:])
                            else:
                                ybst = bc_pool.tile([64, I_BLK], edt,
                                                    tag="ybst")
                                nc.vector.tensor_mul(ybst[:],
                                                     yraws[1][:, isl],
                                                     bcast[:])
                                nc.gpsimd.dma_start(out=yT[64:128, p, isl],
                                                    in_=ybst[:])

        # ---------------- phase 3: output projection ----------------
        with tc.tile_pool(name="ph3o", bufs=3) as ph3o, \
             tc.tile_pool(name="pp_o", bufs=3, space="PSUM") as pp_o:
            for tcl in range(NTC):
                ps = pp_o.tile([128, C], F32, tag="pso")
                for d in range(N_PAIR):
                    for n2 in range(C // 512):
                        nsl = slice(n2 * 512, (n2 + 1) * 512)
                        nc.tensor.matmul(
                            ps[:, nsl],
                            yT[:, d, tcl * 128:(tcl + 1) * 128],
                            wp_sb[:, d, nsl],
                            start=(d == 0), stop=(d == N_PAIR - 1))
                os = ph3o.tile([128, C], F32, tag="os")
                # absorb the WAR wait on the slot's previous out-DMA
                nc.vector.memset(os[0:1, 0:1], 0.0)
                nc.vector.tensor_copy(os[:], ps[:])
                nc.sync.dma_start(out=out[tcl * 128:(tcl + 1) * 128, :],
                                  in_=os[:])

    nc.compile()
    return nc


def make_in_maps(x, w_attn, b_attn, w_proj, T=T_FULL, use_bf16=True):
    """Host-side sharding: per-core input dict."""
    import ml_dtypes
    mdt = ml_dtypes.bfloat16 if use_bf16 else np.float32
    x = np.asarray(x, dtype=np.float32)
    w_attn = np.asarray(w_attn, dtype=np.float32)
    b_attn = np.asarray(b_attn, dtype=np.float32)
    w_proj = np.asarray(w_proj, dtype=np.float32)
    in_maps = []
    ones = np.ones((128, 64), dtype=np.float32)
    for core in range(N_CORES):
        b, g = core // 2, core % 2
        gq = slice(g * QKD, (g + 1) * QKD)
        gk = slice(C + g * QKD, C + (g + 1) * QKD)
        gv = slice(2 * C + g * QKD, 2 * C + (g + 1) * QKD)
        w_qk = np.concatenate([w_attn[:, gq], w_attn[:, gk]], axis=1)
        b_q = b_attn[gq]
        b_k = b_attn[gk]
        b_v = b_attn[gv]
        b_qk = np.stack([b_q.reshape(4, 128), b_k.reshape(4, 128)],
                        axis=0).reshape(8, 128).T.copy()   # [128, 8]
        in_maps.append({
            "xT": np.ascontiguousarray(x[b, :T].T).astype(mdt),
            "w_qk": np.ascontiguousarray(w_qk).astype(mdt),
            "w_v": np.ascontiguousarray(w_attn[:, gv]).astype(mdt),
            "b_qk": np.ascontiguousarray(b_qk),
            "b_v_bc": np.tile(b_v, (128, 1)),
            "w_pr": np.ascontiguousarray(w_proj[gq]).astype(mdt),
            "ones": ones,
        })
    return in_maps


def kernel(x, w_attn, b_attn, w_proj, b_proj):
    global LAST_RESULTS
    in_maps = make_in_maps(x, w_attn, b_attn, w_proj)
    nc = build_bass()
    try:
        res = run_bass_kernel_spmd(nc, in_maps, list(range(N_CORES)),
                                   trace=TRACE)
    except Exception:
        # rare transient NRT exec-unit errors: one retry
        res = run_bass_kernel_spmd(nc, in_maps, list(range(N_CORES)),
                                   trace=TRACE)
    LAST_RESULTS = res
    b_proj = np.asarray(b_proj, dtype=np.float32)
    out = np.empty((B, T_FULL, C), dtype=np.float32)
    for b in range(B):
        out[b] = res.results[2 * b]["out"] + res.results[2 * b + 1]["out"] \
            + b_proj[None, :]
    return out


# revision 33
# speedup vs baseline: 1.0474x; 1.0016x over previous
"""Multi-head attention (no mask) on 8 trn2 NeuronCores.

Problem: x[4,2048,1024] @ w_attn[1024,3072] + b_attn -> qkv, 16 heads x 64,
softmax(q k^T / 8) v, merge heads, @ w_proj[1024,1024] + b_proj.

Sharding: core c = (batch b = c//2, head-group g = c%2).  Each core handles
one batch and 8 heads (tensor-parallel over heads), producing a partial
c_proj output; the host adds the two partials per batch plus b_proj.

Device layout (all fp32):
  xT   [C, T]     host-pretransposed activation (c on partitions on chip)
  qT,kT[512, T]   = (x @ w_q/k + b)^T, stored as 4 chunks of [128, T]
                   (each chunk = 2 heads stacked 64+64 on partitions)
  v    [T, 512]   natural layout, 16 chunks of [128, 512]
  S^T  [j, i]     per head via row-tiled matmuls (k^T stationary)
  exp  on ACT with fused 1/8 scale, no max subtraction (scores are O(5))
  den  = sum_j exp via ones-matmuls (col-tiled into 32-row PSUM strips)
  y^T  [d, i]     via v-stationary col-tiled matmuls (2 heads concurrent)
  out  [T, 1024]  = y^T.T @ w_proj chunks, accumulated over 4 dcat chunks
"""

import numpy as np
from contextlib import ExitStack

import concourse.bass as bass
import concourse.tile as tile
from concourse import bacc, mybir
from concourse.bass_utils import run_bass_kernel_spmd

F32 = mybir.dt.float32
EXP = mybir.ActivationFunctionType.Exp

B, T_FULL, C = 4, 2048, 1024
N_HEAD, HEAD_DIM = 16, 64
HPG = 8           # heads per group (per core)
QKD = HPG * HEAD_DIM   # 512: per-core q/k/v width
N_CORES = 8
SCALE = 1.0 / np.sqrt(HEAD_DIM)

# test.py can flip these to get a profile out of the run
TRACE = False
LAST_RESULTS = None


def build_bass(T=T_FULL, use_bf16=True):
    """Build the per-core Bass program (same program for all 8 cores)."""
    NCC = C // 128          # 8 c-chunks
    NTC = T // 128          # t-chunks (16 at full size)
    TH_SIZE = T // 2        # phase-1 t-half
    NI_TH = TH_SIZE // 512 if TH_SIZE >= 512 else 1   # 512-col mm splits
    I_BLK = min(512, T)
    N_I = T // I_BLK        # i-blocks (4 at full size)
    N_PAIR = HPG // 2       # 4 head pairs

    # Bacc (not raw Bass): its compile() runs generate_event_semaphores,
    # which legalizes multi-wait instructions (HW allows 1 wait/inst).
    nc = bacc.Bacc("TRN2", target_bir_lowering=False, debug=False,
                   num_devices=N_CORES)

    mdt = mybir.dt.bfloat16 if use_bf16 else F32
    xT = nc.dram_tensor("xT", [C, T], mdt, kind="ExternalInput").ap()
    w_qk = nc.dram_tensor("w_qk", [C, 2 * QKD], mdt, kind="ExternalInput").ap()
    w_v = nc.dram_tensor("w_v", [C, QKD], mdt, kind="ExternalInput").ap()
    b_qk = nc.dram_tensor("b_qk", [128, 8], F32, kind="ExternalInput").ap()
    b_v_bc = nc.dram_tensor("b_v_bc", [128, QKD], F32, kind="ExternalInput").ap()
    w_pr = nc.dram_tensor("w_pr", [QKD, C], mdt, kind="ExternalInput").ap()
    ones = nc.dram_tensor("ones", [128, 64], F32, kind="ExternalInput").ap()
    out = nc.dram_tensor("out", [T, C], F32, kind="ExternalOutput").ap()
    # DRAM bounce for softmax reciprocals (SBUF sources cannot
    # partition-broadcast, DRAM sources can)
    N_I_ = T // min(512, T)
    rcd = nc.dram_tensor("rc_scratch", [HPG // 2, 2 * N_I_, min(512, T)],
                         F32).ap()

    BF = mybir.dt.bfloat16
    edt = BF if use_bf16 else F32

    with tile.TileContext(nc) as tc, ExitStack() as ctx:
        persist = ctx.enter_context(tc.tile_pool(name="persist", bufs=1))
        qT = persist.tile([128, N_PAIR, T], edt)
        kT = persist.tile([128, N_PAIR, T], edt)
        # v stored 65-wide per head: 64 data cols + a ones column that
        # makes row 64 of each y matmul the softmax denominator
        v = persist.tile([128, NTC, HPG * 65], edt)
        ones_sb = persist.tile([128, 64], edt)
        bqk_sb = persist.tile([128, 8], F32)
        bvbc_sb = persist.tile([128, QKD], F32)

        dvescr = persist.tile([1, 8], F32)
        nc.sync.dma_start(out=bqk_sb[:], in_=b_qk)
        nc.sync.dma_start(out=bvbc_sb[:], in_=b_v_bc)
        # DVE-side fences: TT/TS instructions also hold only one sync
        # wait, so absorb each bias-DMA wait into a tiny copy first
        nc.vector.tensor_copy(dvescr[0:1, 0:1], bqk_sb[0:1, 0:1])
        nc.vector.tensor_copy(dvescr[0:1, 1:2], bvbc_sb[0:1, 0:1])
        if use_bf16:
            nc.gpsimd.dma_start(out=ones_sb[:], in_=ones)  # casts f32->bf16
            ones_f32 = persist.tile([128, 64], F32)
            nc.sync.dma_start(out=ones_f32[:], in_=ones)
        else:
            nc.sync.dma_start(out=ones_sb[:], in_=ones)
            ones_f32 = ones_sb

        yT = persist.tile([128, N_PAIR, T], edt)
        wp_sb = persist.tile([128, N_PAIR, C], edt)

        # ---------------- phase 1a: loads + v projection ----------------
        # Fences: each freshly-DMA'd matmul input gets a 1x1x1 dummy
        # matmul so real matmuls see at most one unobserved semaphore
        # (keeps Bacc's event-semaphore splitting to a minimum).
        xT_r = xT.rearrange("(c p) t -> p c t", p=128)
        with tc.tile_pool(name="ph1w", bufs=1) as ph1w, \
             tc.tile_pool(name="ph1x", bufs=2) as ph1x:
            xt0 = ph1x.tile([128, NCC, TH_SIZE], edt, tag="xt")
            xt1 = ph1x.tile([128, NCC, TH_SIZE], edt, tag="xt")
            wqk_sb = ph1w.tile([128, NCC, 2 * QKD], edt)
            wv_sb = ph1w.tile([128, NCC, QKD], edt)
            # pair-0's q/k weight slices land first so the first
            # projection matmuls (and hence exp) start ~25us earlier
            wqk_r = w_qk.rearrange("(c p) n -> p c n", p=128)

            def load_wqk_pair(pp):
                for dcq in (pp, pp + 4):
                    csl = slice(dcq * 128, (dcq + 1) * 128)
                    nc.sync.dma_start(out=wqk_sb[:, :, csl],
                                      in_=wqk_r[:, :, csl])

            load_wqk_pair(0)
            nc.sync.dma_start(out=xt0[:], in_=xT_r[:, :, 0:TH_SIZE])
            nc.sync.dma_start(out=xt1[:], in_=xT_r[:, :, TH_SIZE:T])
            nc.sync.dma_start(out=wv_sb[:],
                              in_=w_v.rearrange("(c p) n -> p c n", p=128))
            for pp in range(1, N_PAIR):
                load_wqk_pair(pp)
            nc.sync.dma_start(out=wp_sb[:],
                              in_=w_pr.rearrange("(d p) n -> p d n", p=128))
            xts = [xt0, xt1]
            v_r = v[:, :, :].rearrange("q t (h e) -> q t h e", e=65)
            nc.vector.memset(v_r[:, :, :, 64:65], 1.0)

            # v projection is emitted per-tc inside pair 0's first
            # j-loop (tc == j there), so exp can start ~30us earlier.
            def v_proj_tc(tg, ps_pool):
                psv = ps_pool.tile([128, QKD], F32, tag="ps")
                th = tg // (TH_SIZE // 128)
                tcl = tg % (TH_SIZE // 128)
                for c in range(NCC):
                    nc.tensor.matmul(
                        psv[:],
                        xts[th][:, c, tcl * 128:(tcl + 1) * 128],
                        wv_sb[:, c, :],
                        start=(c == 0), stop=(c == NCC - 1))
                nc.vector.tensor_add(
                    v[:, tg, :].rearrange(
                        "q (h e) -> q h e", e=65)[:, :, 0:64],
                    psv[:].rearrange("q (h e) -> q h e", e=64),
                    bvbc_sb[:].rearrange("q (h e) -> q h e", e=64))

            # ------- phase 1b/2: per-pair q/k projection + attention -------
            # Emitting each pair's q/k projection right before its
            # attention lets the scheduler fill PE idle slots (attention
            # is exp/ACT-paced) with the next pair's projection matmuls.
            def qk_proj(p, qk_pool):
                for dcq in (p, p + 4):
                    for th in range(2):
                        for i2 in range(NI_TH):
                            lo = th * TH_SIZE + i2 * 512
                            w = min(512, TH_SIZE)
                            isl = slice(i2 * 512, i2 * 512 + w)
                            ps = qk_pool.tile([128, 512], F32, tag="ps")
                            for c in range(NCC):
                                nc.tensor.matmul(
                                    ps[:, 0:w],
                                    wqk_sb[:, c, dcq * 128:(dcq + 1) * 128],
                                    xts[th][:, c, isl],
                                    start=(c == 0), stop=(c == NCC - 1))
                            dst = (qT if dcq < 4 else kT)[:, p, lo:lo + w]
                            nc.vector.tensor_scalar_add(
                                dst, ps[:, 0:w], bqk_sb[:, dcq:dcq + 1])

        # ---------------- phase 2: attention ----------------
        # Per head: S^T via K=64 matmuls (row-pair per es grain), then
        # y accumulation with M=65 single-tile matmuls whose 65th lhsT
        # column is all-ones -> row 64 of the y accumulator is the
        # softmax denominator (free: matmul time is N-bound).
            with tc.tile_pool(name="pp_qk", bufs=2, space="PSUM") as qk_pool, \
                 tc.tile_pool(name="att_s", bufs=2, space="PSUM") as s_pool, \
                 tc.tile_pool(name="att_y", bufs=2, space="PSUM") as y_pool, \
                 tc.tile_pool(name="att_es", bufs=6) as es_pool, \
                 tc.tile_pool(name="att_yr", bufs=4) as yr_pool, \
                 tc.tile_pool(name="att_st", bufs=6) as st_pool, \
                 tc.tile_pool(name="att_cl", bufs=3) as cl_pool, \
                 tc.tile_pool(name="att_rc", bufs=3) as rc_pool, \
                 tc.tile_pool(name="att_bc", bufs=4) as bc_pool:
                fence_ps = y_pool.tile([1, 8], F32, tag="y")
                nc.tensor.matmul(fence_ps[0:1, 0:1], ones_sb[0:1, 0:1],
                                 ones_sb[0:1, 0:1], start=True, stop=True)
                if use_bf16:
                    nc.tensor.matmul(fence_ps[0:1, 1:2], ones_f32[0:1, 0:1],
                                     ones_f32[0:1, 0:1], start=True, stop=True)
                for fi, ft in enumerate((xt0, wv_sb, xt1, wqk_sb, wp_sb)):
                    nc.tensor.matmul(fence_ps[0:1, 2 + fi:3 + fi],
                                     ft[0:1, 0, 0:1], ones_sb[0:1, 0:1],
                                     start=True, stop=True)
                for p in range(N_PAIR):
                    qk_proj(p, qk_pool)
                    yraw_a = yr_pool.tile([64, T], edt, tag="yraw")
                    yraw_b = yr_pool.tile([64, T], edt, tag="yraw")
                    yraws = [yraw_a, yraw_b]
                    for i in range(N_I):
                        isl = slice(i * I_BLK, (i + 1) * I_BLK)
                        y_a = y_pool.tile([65, I_BLK], F32, tag="y")
                        y_b = y_pool.tile([65, I_BLK], F32, tag="y")
                        ys = [y_a, y_b]
                        for j in range(NTC):
                            jsl = slice(j * 128, (j + 1) * 128)
                            s = s_pool.tile([128, 2 * I_BLK], F32, tag="s")
                            nc.tensor.matmul(s[:, 0:I_BLK],
                                             kT[0:64, p, jsl], qT[0:64, p, isl],
                                             start=True, stop=True)
                            nc.tensor.matmul(s[:, I_BLK:2 * I_BLK],
                                             kT[64:128, p, jsl],
                                             qT[64:128, p, isl],
                                             start=True, stop=True)
                            if p == 0 and i == 0:
                                v_proj_tc(j, qk_pool)
                            es = es_pool.tile([128, 2 * I_BLK], edt, tag="es")
                            nc.scalar.activation(es[:], s[:], EXP, scale=SCALE)
                            for hl in range(2):
                                h = 2 * p + hl
                                nc.tensor.matmul(
                                    ys[hl][0:65, :],
                                    v[:, j, 65 * h:65 * h + 65],
                                    es[:, hl * I_BLK:(hl + 1) * I_BLK],
                                    start=(j == 0), stop=(j == NTC - 1))
                        # per-i denominator handling so yT streams out
                        # (keeps the output projection from piling up at
                        # the very end of attention)
                        coll = cl_pool.tile([2, I_BLK], F32, tag="coll")
                        for hl in range(2):
                            st = st_pool.tile([65, I_BLK], F32, tag="st")
                            nc.vector.tensor_copy(st[64:65, :],
                                                  ys[hl][64:65, :])
                            nc.vector.tensor_copy(yraws[hl][:, isl],
                                                  ys[hl][0:64, :])
                            # move denominator row to its own partition
                            nc.gpsimd.dma_start(out=coll[hl:hl + 1, :],
                                                in_=st[64:65, :])
                        rc = rc_pool.tile([2, I_BLK], F32, tag="rc")
                        nc.vector.reciprocal_approx_fast(rc[:], coll[:])
                        nc.gpsimd.dma_start(out=rcd[p, 2 * i:2 * i + 2],
                                            in_=rc[:])
                        for hl in range(2):
                            bcast = bc_pool.tile([64, I_BLK], edt, tag="bcast")
                            rrow = rcd[p, 2 * i + hl, :]
                            rbc = bass.AP(tensor=rrow.tensor,
                                          offset=rrow.offset,
                                          ap=[[0, 64]] + list(rrow.ap))
                            nc.gpsimd.dma_start(out=bcast[:], in_=rbc)
                            if hl == 0:
                                nc.vector.tensor_mul(yT[0:64, p, isl],
                                                     yraws[0][:, isl],
                                                     bcast[:])
                            else:
                                ybst = bc_pool.tile([64, I_BLK], edt,
                                                    tag="ybst")
                                nc.vector.tensor_mul(ybst[:],
                                                     yraws[1][:, isl],
                                                     bcast[:])
                                nc.gpsimd.dma_start(out=yT[64:128, p, isl],
                                                    in_=ybst[:])

        # ---------------- phase 3: output projection ----------------
        with tc.tile_pool(name="ph3o", bufs=3) as ph3o, \
             tc.tile_pool(name="pp_o", bufs=3, space="PSUM") as pp_o:
            for tcl in range(NTC):
                ps = pp_o.tile([128, C], F32, tag="pso")
                for d in range(N_PAIR):
                    for n2 in range(C // 512):
                        nsl = slice(n2 * 512, (n2 + 1) * 512)
                        nc.tensor.matmul(
                            ps[:, nsl],
                            yT[:, d, tcl * 128:(tcl + 1) * 128],
                            wp_sb[:, d, nsl],
                            start=(d == 0), stop=(d == N_PAIR - 1))
                os = ph3o.tile([128, C], F32, tag="os")
                # absorb the WAR wait on the slot's previous out-DMA
                nc.vector.memset(os[0:1, 0:1], 0.0)
                nc.vector.tensor_copy(os[:], ps[:])
                nc.sync.dma_start(out=out[tcl * 128:(tcl + 1) * 128, :],
                                  in_=os[:])

    nc.compile()
    return nc


def make_in_maps(x, w_attn, b_attn, w_proj, T=T_FULL, use_bf16=True):
    """Host-side sharding: per-core input dict."""
    import ml_dtypes
    mdt = ml_dtypes.bfloat16 if use_bf16 else np.float32
    x = np.asarray(x, dtype=np.float32)
    w_attn = np.asarray(w_attn, dtype=np.float32)
    b_attn = np.asarray(b_attn, dtype=np.float32)
    w_proj = np.asarray(w_proj, dtype=np.float32)
    in_maps = []
    ones = np.ones((128, 64), dtype=np.float32)
    for core in range(N_CORES):
        b, g = core // 2, core % 2
        gq = slice(g * QKD, (g + 1) * QKD)
        gk = slice(C + g * QKD, C + (g + 1) * QKD)
        gv = slice(2 * C + g * QKD, 2 * C + (g + 1) * QKD)
        w_qk = np.concatenate([w_attn[:, gq], w_attn[:, gk]], axis=1)
        b_q = b_attn[gq]
        b_k = b_attn[gk]
        b_v = b_attn[gv]
        b_qk = np.stack([b_q.reshape(4, 128), b_k.reshape(4, 128)],
                        axis=0).reshape(8, 128).T.copy()   # [128, 8]
        in_maps.append({
            "xT": np.ascontiguousarray(x[b, :T].T).astype(mdt),
            "w_qk": np.ascontiguousarray(w_qk).astype(mdt),
            "w_v": np.ascontiguousarray(w_attn[:, gv]).astype(mdt),
            "b_qk": np.ascontiguousarray(b_qk),
            "b_v_bc": np.tile(b_v, (128, 1)),
            "w_pr": np.ascontiguousarray(w_proj[gq]).astype(mdt),
            "ones": ones,
        })
    return in_maps


def kernel(x, w_attn, b_attn, w_proj, b_proj):
    global LAST_RESULTS
    in_maps = make_in_maps(x, w_attn, b_attn, w_proj)
    nc = build_bass()
    try:
        res = run_bass_kernel_spmd(nc, in_maps, list(range(N_CORES)),
                                   trace=TRACE)
    except Exception:
        # rare transient NRT exec-unit errors: one retry
        res = run_bass_kernel_spmd(nc, in_maps, list(range(N_CORES)),
                                   trace=TRACE)
    LAST_RESULTS = res
    b_proj = np.asarray(b_proj, dtype=np.float32)
    out = np.empty((B, T_FULL, C), dtype=np.float32)
    for b in range(B):
        out[b] = res.results[2 * b]["out"] + res.results[2 * b + 1]["out"] \
            + b_proj[None, :]
    return out
